# revision 18
# baseline (speedup 1.0000x reference)
"""Trainium2 Bass kernel for nn_LogicReasoningEncoder (GNN message passing).

Sharding: 8 cores = 4 batches x 2 target-node halves. Each core owns the
edges whose target node falls in its half, bucketed into 8 fixed-capacity
node blocks of 128 nodes so that every core runs the identical SPMD program.
Scatter-softmax is reformulated without the max pass (att is leaky-relu
bounded, so exp never overflows) and the alpha normalization is deferred to
a per-node divide after aggregation, so cross-core work is just one
pair-wise AllGather of updated node features per layer.
"""

import sys
import numpy as np

for _p in ("/opt/trn_rl_repo", "/root/.axon_site/_ro/trn_rl_repo"):
    if _p not in sys.path:
        sys.path.append(_p)

import concourse.bass as bass
import concourse.mybir as mybir
from concourse import bacc, tile
from concourse.bass_utils import run_bass_kernel_spmd

F32 = mybir.dt.float32
BF16 = mybir.dt.bfloat16
I16 = mybir.dt.int16
BF16_NP = mybir.dt.np(BF16)

B, N, E, D, L, NREL, TAU = 4, 2048, 32768, 128, 3, 1000, 0.1
NHALF = N // 2          # nodes per core
NBLK = NHALF // 128     # 8 node blocks per core
P = 128


# ----------------------------------------------------------------------------
# Host-side sharding / layout prep (index manipulation + layout only)
# ----------------------------------------------------------------------------

def _chunkify(x, ecap):
    """[ecap] -> [128, ecap//128] with x[c*128+p] at [p, c]."""
    return np.ascontiguousarray(x.reshape(ecap // 128, 128).T)


def _wrap16(x, ecap):
    """[ecap] -> int16 [128, ecap//16]: x[i] at [i%16, i//16], tiled x8 for Q7 cores."""
    w = np.ascontiguousarray(x.astype(np.int16).reshape(ecap // 16, 16).T)
    return np.ascontiguousarray(np.tile(w, (8, 1)))


def prepare_core_inputs(inputs):
    ei = np.asarray(inputs["edge_index"])          # [B, 2, E] int32
    rels = np.asarray(inputs["rels"])              # [B, E]
    scores = np.asarray(inputs["scores"])          # [B, E] f32
    cm = np.asarray(inputs["edge_conf_mask"])      # [B, E] bool
    em = np.asarray(inputs["edge_mask"])           # [B, E] bool
    conf = np.asarray(inputs["conf_embeds"])       # [B, E, D] f32

    # Fixed per-block edge capacity, uniform across all cores (SPMD).
    max_cnt = 0
    per_core = []
    for core in range(8):
        b, half = core // 2, core % 2
        base = half * NHALF
        tgt = ei[b, 1]
        sel = np.nonzero((tgt >= base) & (tgt < base + NHALF))[0]
        ltgt = tgt[sel] - base
        blk = ltgt >> 7
        cnts = np.bincount(blk, minlength=NBLK)
        max_cnt = max(max_cnt, int(cnts.max()))
        per_core.append((b, base, sel, ltgt, blk))

    e_blk = ((max_cnt + 383) // 384) * 384
    ecap = NBLK * e_blk

    h0 = np.zeros((N, D), dtype=BF16_NP)
    h0[0, :] = 1.0

    in_maps = []
    for core in range(8):
        b, base, sel, ltgt, blk = per_core[core]
        order = np.argsort(blk, kind="stable")
        perm = sel[order]                 # original edge ids, block-sorted
        lt = ltgt[order]
        bk = blk[order]
        cnts = np.bincount(bk, minlength=NBLK)
        slot = np.concatenate(
            [bb * e_blk + np.arange(cnts[bb]) for bb in range(NBLK)]
        ).astype(np.int64) if len(perm) else np.zeros(0, np.int64)

        src_p = np.zeros(ecap, np.int32)
        rels_p = np.zeros(ecap, np.int32)
        scores_p = np.zeros(ecap, np.float32)
        em_p = np.zeros(ecap, np.float32)
        cm_p = np.zeros(ecap, np.float32)
        conf_p = np.zeros((ecap, D), np.float32)
        oh = np.zeros((ecap, P), np.float32)

        src_p[slot] = ei[b, 0][perm]
        rels_p[slot] = rels[b][perm]
        scores_p[slot] = scores[b][perm]
        em_p[slot] = em[b][perm].astype(np.float32)
        cm_p[slot] = cm[b][perm].astype(np.float32)
        conf_p[slot] = conf[b][perm]
        j = lt - (slot // e_blk) * 128
        oh[slot, j] = 1.0
        # partition-major one-hot: [p, c*128 + j]
        oh_pm = np.ascontiguousarray(
            oh.reshape(ecap // 128, 128, 128).transpose(1, 0, 2).reshape(P, ecap)
        ).astype(BF16_NP)

        hown = np.zeros((P, NBLK * 128), np.float32)
        if core % 2 == 0:
            hown[0, 0:128] = 1.0  # node 0 lives at block 0, partition 0

        m = {
            "conf_ed": np.ascontiguousarray(conf_p),
            "onehot_pm": oh_pm,
            "srcz_row": (src_p == 0).astype(BF16_NP),
            "scores_ch": _chunkify(scores_p, ecap),
            "em_ch": _chunkify(em_p, ecap),
            "cm_ch": _chunkify(cm_p, ecap),
            "src_wr": _wrap16(src_p, ecap),
            "rels_wr": _wrap16(rels_p, ecap),
            "h0": h0,
            "hown0": hown,
            "ident_f": np.eye(P, dtype=np.float32),
            "ident_b": np.eye(P).astype(BF16_NP),
            "rq": np.asarray(inputs["r_query_embed"])[b].reshape(D, 1).astype(np.float32),
            "rel_table": np.asarray(inputs["rel_table"]).astype(np.float32),
            "beta_W": np.asarray(inputs["beta_W"]).astype(np.float32),
            "msg_W": np.asarray(inputs["msg_W"]).astype(np.float32),
            "msg_b_col": np.ascontiguousarray(np.asarray(inputs["msg_b"]).T).astype(np.float32),
            "upd_W": np.asarray(inputs["upd_W"]).astype(np.float32),
            "upd_b_row": np.asarray(inputs["upd_b"]).reshape(L, D).astype(np.float32),
            "ln_g_row": np.asarray(inputs["ln_g"]).reshape(1, D).astype(np.float32),
            "ln_b_row": np.asarray(inputs["ln_b"]).reshape(1, D).astype(np.float32),
            "att_W": np.asarray(inputs["att_W"]).astype(np.float32),
            "sc_bias": np.concatenate([
                np.asarray(inputs["att_b"]).reshape(-1),       # 3
                np.asarray(inputs["beta_b"]).reshape(-1),      # 1
                np.asarray(inputs["den_b2"]).reshape(-1),      # 1
                np.zeros(1, np.float32),
            ]).reshape(1, 6).astype(np.float32),
            "den_W1": np.asarray(inputs["den_W1"]).astype(np.float32),
            "den_b1_row": np.asarray(inputs["den_b1"]).reshape(1, D).astype(np.float32),
            "den_W2": np.asarray(inputs["den_W2"]).astype(np.float32),
        }
        in_maps.append(m)
    return in_maps, ecap


# ----------------------------------------------------------------------------
# Device program
# ----------------------------------------------------------------------------

def build_program(ecap):
    C = ecap // 128          # chunks
    T = ecap // 512          # 512-edge tiles
    SLAB = 3072
    NSLAB = ecap // SLAB     # slabs (6 tiles each)
    ST = SLAB // 512         # tiles per slab (6)
    SC = SLAB // 128         # chunks per slab (24)
    CPB = (ecap // NBLK) // 128  # chunks per node block

    nc = bacc.Bacc("TRN2", num_devices=8, debug=False)

    dp = nc.declare_dram_parameter
    conf_ed = dp("conf_ed", [ecap, D], F32, isOutput=False)
    onehot_pm = dp("onehot_pm", [P, ecap], BF16, isOutput=False)
    srcz_row = dp("srcz_row", [ecap], BF16, isOutput=False)
    scores_ch_d = dp("scores_ch", [P, C], F32, isOutput=False)
    em_ch_d = dp("em_ch", [P, C], F32, isOutput=False)
    cm_ch_d = dp("cm_ch", [P, C], F32, isOutput=False)
    src_wr_d = dp("src_wr", [128, ecap // 16], I16, isOutput=False)
    rels_wr_d = dp("rels_wr", [128, ecap // 16], I16, isOutput=False)
    h0_d = dp("h0", [N, D], BF16, isOutput=False)
    hown0_d = dp("hown0", [P, NBLK * 128], F32, isOutput=False)
    ident_f_d = dp("ident_f", [P, P], F32, isOutput=False)
    ident_b_d = dp("ident_b", [P, P], BF16, isOutput=False)
    rq_d = dp("rq", [D, 1], F32, isOutput=False)
    rel_table_d = dp("rel_table", [NREL, D], F32, isOutput=False)
    beta_W_d = dp("beta_W", [D, 1], F32, isOutput=False)
    msg_W_d = dp("msg_W", [L, 5 * D, D], F32, isOutput=False)
    msg_b_col_d = dp("msg_b_col", [D, L], F32, isOutput=False)
    upd_W_d = dp("upd_W", [L, D, D], F32, isOutput=False)
    upd_b_row_d = dp("upd_b_row", [L, D], F32, isOutput=False)
    ln_g_row_d = dp("ln_g_row", [1, D], F32, isOutput=False)
    ln_b_row_d = dp("ln_b_row", [1, D], F32, isOutput=False)
    att_W_d = dp("att_W", [L, 3 * D, 1], F32, isOutput=False)
    sc_bias_d = dp("sc_bias", [1, 6], F32, isOutput=False)
    den_W1_d = dp("den_W1", [3 * D, D], F32, isOutput=False)
    den_b1_row_d = dp("den_b1_row", [1, D], F32, isOutput=False)
    den_W2_d = dp("den_W2", [D, 1], F32, isOutput=False)
    out_d = dp("out", [L, D], F32, isOutput=True)

    # DRAM scratch
    rel_bf = nc.dram_tensor("rel_bf", [NREL, D], BF16)
    conf_fm_dr = nc.dram_tensor("conf_fm_dr", [P, ecap], BF16)
    hhalf = nc.dram_tensor("hhalf", [NHALF, D], BF16)
    rows_dr = nc.dram_tensor("rows_dr", [5, ecap], F32)
    att_dr = nc.dram_tensor("att_dr", [ecap], F32)
    hfull = [nc.dram_tensor(f"hfull{i}", [N, D], BF16) for i in range(2)]

    AF = mybir.ActivationFunctionType
    ALU = mybir.AluOpType

    with tile.TileContext(nc) as tc:
        with (
            tc.tile_pool(name="res", bufs=1) as res,
            tc.tile_pool(name="wgt", bufs=1) as wgt,
        ):
            # ---------------- persistent SBUF ----------------
            hr_fm = res.tile([P, ecap], BF16)
            s_ch = res.tile([P, C], F32)
            attrelp = res.tile([P, L, C], F32)
            att_ch = res.tile([P, C], F32)
            w_ch = res.tile([P, C], F32)
            exab_ch = res.tile([P, C], BF16)
            em_ch = res.tile([P, C], F32)
            src_wr = res.tile([128, ecap // 16], I16)
            h_tiles = [res.tile([P, NBLK, 128], F32, name=f"h_t{i}", tag=f"h_t{i}")
                       for i in range(L + 1)]

            # ---------------- weights in SBUF ----------------
            msgW = wgt.tile([P, L, 5, D], BF16)
            denW = wgt.tile([P, 3, D], BF16)      # A, B, C blocks of den_W1
            updW = wgt.tile([P, L, D], BF16)
            a1 = wgt.tile([P, L], BF16)
            attbeta4 = wgt.tile([P, 4], BF16)     # a2_0..2, beta_W
            aq6 = wgt.tile([P, 6], BF16)          # a3_0..2, beta_W, 0, 0
            denW2 = wgt.tile([P, 1], BF16)
            msgb = wgt.tile([P, L], F32)
            w3sum = wgt.tile([1, L, D], BF16)
            updb_row = wgt.tile([1, L, D], BF16)
            denb1_row = wgt.tile([1, D], BF16)
            rq_bf = wgt.tile([P, 1], BF16)
            scb_bf = wgt.tile([1, 6], BF16)
            ident_f = wgt.tile([P, P], F32)
            ident_b = wgt.tile([P, P], BF16)
            ones_col = wgt.tile([P, 1], BF16)
            ones_r1b = wgt.tile([1, P], BF16)
            ones_r1f = wgt.tile([1, P], F32)
            ones11 = wgt.tile([1, 1], BF16)
            eps_col = wgt.tile([P, 1], F32)
            g_rep = wgt.tile([P, P], F32)
            b_rep = wgt.tile([P, P], F32)
            rep6 = wgt.tile([P, 6], F32)
            den_bias = wgt.tile([P, 1], F32)
            row6_bf = wgt.tile([1, 6], BF16)

            gp, sy, ve, sc, te = nc.gpsimd, nc.sync, nc.vector, nc.scalar, nc.tensor

            # ---------------- step 0: load + cast weights ----------------
            gp.dma_start(msgW[:], msg_W_d[:].rearrange("k (t i) o -> i k t o", i=P))
            gp.dma_start(denW[:], den_W1_d[:].rearrange("(t i) o -> i t o", i=P))
            gp.dma_start(updW[:], upd_W_d[:].rearrange("k i o -> i k o"))
            for k in range(L):
                gp.dma_start(a1[:, k:k + 1], att_W_d[k, 0:P, :])
                gp.dma_start(attbeta4[:, k:k + 1], att_W_d[k, P:2 * P, :])
                gp.dma_start(aq6[:, k:k + 1], att_W_d[k, 2 * P:3 * P, :])
            gp.dma_start(attbeta4[:, 3:4], beta_W_d[:])
            gp.dma_start(aq6[:, 3:4], beta_W_d[:])
            ve.memset(aq6[:, 4:6], 0.0)
            gp.dma_start(denW2[:], den_W2_d[:])
            sy.dma_start(msgb[:], msg_b_col_d[:])
            gp.dma_start(updb_row[:], upd_b_row_d[:].rearrange("k d -> () k d"))
            gp.dma_start(denb1_row[:], den_b1_row_d[:])
            gp.dma_start(rq_bf[:], rq_d[:])
            gp.dma_start(scb_bf[:], sc_bias_d[:])
            sy.dma_start(ident_f[:], ident_f_d[:])
            sy.dma_start(ident_b[:], ident_b_d[:])
            ve.memset(ones_col[:], 1.0)
            ve.memset(ones_r1b[:], 1.0)
            ve.memset(ones_r1f[:], 1.0)
            ve.memset(ones11[:], 1.0)
            ve.memset(eps_col[:], 1e-5)
            sy.dma_start(em_ch[:], em_ch_d[:])
            sy.dma_start(src_wr[:], src_wr_d[:])
            sy.dma_start(h_tiles[0][:].rearrange("p b d -> p (b d)"), hown0_d[:])

            # rel table -> bf16 in DRAM (for the transposed gather)
            with tc.tile_pool(name="prep0", bufs=1) as pp0:
                relsb = pp0.tile([125, 8, D], BF16)
                gp.dma_start(relsb[:], rel_table_d[:].rearrange("(a p) d -> p a d", p=125))
                gp.dma_start(rel_bf[:].rearrange("(a p) d -> p a d", p=125), relsb[:])

            with tc.tile_pool(name="prep_ps", bufs=1, space="PSUM") as pps:
                # w3sum_k = ones^T @ W3_k
                w3ps = pps.tile([1, L, D], F32)
                for k in range(L):
                    te.matmul(w3ps[:, k, :], ones_col[:], msgW[:, k, 2, :])
                sc.copy(w3sum[:], w3ps[:])

                # row6 = rq^T @ [a3_0,a3_1,a3_2,beta_W,0,0] + sc_bias
                r6ps = pps.tile([1, 6], F32)
                te.matmul(r6ps[:], rq_bf[:], aq6[:], start=True, stop=False)
                te.matmul(r6ps[:], ones11[:], scb_bf[:], start=False, stop=True)
                sc.copy(row6_bf[:], r6ps[:])

                # rep6 = ones ⊗ row6 ; den_bias = denB^T rq + den_b1
                rp6 = pps.tile([P, 6], F32)
                te.matmul(rp6[:], ones_r1b[:], row6_bf[:])
                ve.tensor_copy(rep6[:], rp6[:])

                dbp = pps.tile([P, 1], F32)
                te.matmul(dbp[:], denW[:, 1, :], rq_bf[:], start=True, stop=False)
                te.matmul(dbp[:], denb1_row[:], ones11[:], start=False, stop=True)
                ve.tensor_copy(den_bias[:], dbp[:])

                # g_rep / b_rep (fp32 broadcast matmuls)
                lng = pp0_row = wgt.tile([1, D], F32, name="lng_row")
                lnb = wgt.tile([1, D], F32, name="lnb_row")
                sy.dma_start(lng[:], ln_g_row_d[:])
                sy.dma_start(lnb[:], ln_b_row_d[:])
                grp = pps.tile([P, D], F32)
                te.matmul(grp[:], ones_r1f[:], lng[:])
                ve.tensor_copy(g_rep[:], grp[:])
                brp = pps.tile([P, D], F32)
                te.matmul(brp[:], ones_r1f[:], lnb[:])
                ve.tensor_copy(b_rep[:], brp[:])

            # ---------------- phase A: h_r gather, conf transpose, den gate ----
            with (
                tc.tile_pool(name="pA", bufs=2) as pA,
                tc.tile_pool(name="pA3", bufs=3) as pA3,
                tc.tile_pool(name="pA_ps", bufs=2, space="PSUM") as pAps,
                tc.tile_pool(name="chA", bufs=1) as chA,
            ):
                rels_wr = chA.tile([128, ecap // 16], I16)
                sy.dma_start(rels_wr[:], rels_wr_d[:])
                scores_ch = chA.tile([P, C], F32)
                cm_ch = chA.tile([P, C], F32)
                betarel_ch = chA.tile([P, C], F32)
                denlin_ch = chA.tile([P, C], F32)
                sy.dma_start(scores_ch[:], scores_ch_d[:])
                sy.dma_start(cm_ch[:], cm_ch_d[:])

                for s in range(NSLAB):
                    lo = s * SLAB
                    gp.dma_gather(
                        hr_fm[:, lo:lo + SLAB].rearrange("p (o e) -> p o e", o=1),
                        rel_bf[:],
                        rels_wr[:, lo // 16:(lo + SLAB) // 16],
                        SLAB, SLAB, D, transpose=True, single_packet=False,
                    )

                for s in range(NSLAB):
                    r4_sl = pA.tile([4, SLAB], F32, tag="r4sl")
                    dl_sl = pA.tile([1, SLAB], F32, tag="dlsl")
                    for tt in range(ST):
                        t = s * ST + tt
                        e0 = t * 512
                        ce = pA3.tile([P, 4, P], F32, tag="ce")
                        sy.dma_start(
                            ce[:], conf_ed[:].rearrange("(c p) d -> p c d", p=P)[:, 4 * t:4 * t + 4, :]
                        )
                        trp = pAps.tile([P, 4, P], F32, tag="trA")
                        for j in range(4):
                            te.transpose(trp[:, j, :], ce[:, j, :], ident_f[:])
                        cf = pA3.tile([P, 512], BF16, tag="cf")
                        sc.copy(cf[:], trp[:].rearrange("p a d -> p (a d)"))
                        sy.dma_start(conf_fm_dr[:, e0:e0 + 512], cf[:])

                        dps = pAps.tile([P, 512], F32, tag="denps")
                        te.matmul(dps[:], denW[:, 0, :], hr_fm[:, e0:e0 + 512],
                                  start=True, stop=False)
                        te.matmul(dps[:], denW[:, 2, :], cf[:], start=False, stop=True)
                        hid = pA3.tile([P, 512], BF16, tag="hid")
                        sc.activation(hid[:], dps[:], AF.Relu, bias=den_bias[:])

                        dlp = pAps.tile([1, 512], F32, tag="dlps")
                        te.matmul(dlp[:], denW2[:], hid[:])
                        sc.copy(dl_sl[:, 512 * tt:512 * tt + 512], dlp[:])

                        r4p = pAps.tile([4, 512], F32, tag="r4ps")
                        te.matmul(r4p[:], attbeta4[:], hr_fm[:, e0:e0 + 512])
                        ve.tensor_copy(r4_sl[:, 512 * tt:512 * tt + 512], r4p[:])

                    c0 = s * SC
                    lo = s * SLAB
                    sy.dma_start(rows_dr[0:4, lo:lo + SLAB], r4_sl[:])
                    sy.dma_start(rows_dr[4:5, lo:lo + SLAB], dl_sl[:])
                    for k in range(L):
                        sy.dma_start(
                            attrelp[:, k, c0:c0 + SC].opt(),
                            rows_dr[k, lo:lo + SLAB].rearrange("(c p) -> p c", p=P),
                        )
                    sy.dma_start(
                        betarel_ch[:, c0:c0 + SC].opt(),
                        rows_dr[3, lo:lo + SLAB].rearrange("(c p) -> p c", p=P),
                    )
                    sy.dma_start(
                        denlin_ch[:, c0:c0 + SC].opt(),
                        rows_dr[4, lo:lo + SLAB].rearrange("(c p) -> p c", p=P),
                    )

                # chunk-layout gate math
                beta_t = chA.tile([P, C], F32)
                sc.activation(beta_t[:], betarel_ch[:], AF.Sigmoid, bias=rep6[:, 3:4])
                tmp_t = chA.tile([P, C], F32)
                ve.tensor_tensor(tmp_t[:], scores_ch[:], beta_t[:], ALU.subtract)
                gk_t = chA.tile([P, C], F32)
                sc.activation(gk_t[:], tmp_t[:], AF.Sigmoid, scale=1.0 / TAU)
                ve.tensor_scalar(gk_t[:], gk_t[:], -0.5, None, ALU.add)
                ve.tensor_tensor(gk_t[:], cm_ch[:], gk_t[:], ALU.mult)
                ve.tensor_scalar(gk_t[:], gk_t[:], 0.5, None, ALU.add)   # gate
                den_t = chA.tile([P, C], F32)
                sc.activation(den_t[:], denlin_ch[:], AF.Sigmoid, bias=rep6[:, 4:5])
                ve.tensor_tensor(s_ch[:], gk_t[:], den_t[:], ALU.mult)
                ve.tensor_tensor(s_ch[:], s_ch[:], em_ch[:], ALU.mult)
                for k in range(L):
                    ve.tensor_scalar(attrelp[:, k, :], attrelp[:, k, :],
                                     rep6[:, k:k + 1], None, ALU.add)

            # ---------------- phase B: layers ----------------
            for k in range(L):
                h_read = h0_d if k == 0 else hfull[(k - 1) % 2]
                with tc.tile_pool(name=f"ups{k}", bufs=1, space="PSUM") as upool:
                  usum_ps = upool.tile([P, NBLK, P], F32)
                  sm_ps = upool.tile([P, NBLK], F32)
                  with (
                    tc.tile_pool(name=f"lps{k}", bufs=1, space="PSUM") as lpool,
                    tc.tile_pool(name=f"sl{k}", bufs=2) as slp,
                    tc.tile_pool(name=f"tp{k}", bufs=3) as tpp,
                    tc.tile_pool(name=f"rm{k}", bufs=8) as rmp,
                  ):
                    for s in range(NSLAB):
                        lo = s * SLAB
                        hsrc_sl = slp.tile([P, SLAB], BF16, tag="hsrc")
                        gp.dma_gather(
                            hsrc_sl[:].rearrange("p (o e) -> p o e", o=1),
                            h_read[:],
                            src_wr[:, lo // 16:(lo + SLAB) // 16],
                            SLAB, SLAB, D, transpose=True, single_packet=False,
                        )
                        conf_sl = slp.tile([P, SLAB], BF16, tag="conf")
                        sy.dma_start(conf_sl[:], conf_fm_dr[:, lo:lo + SLAB])
                        oh_sl = slp.tile([P, SLAB], BF16, tag="oh")
                        sy.dma_start(oh_sl[:], onehot_pm[:, lo:lo + SLAB])
                        srcz_sl = slp.tile([1, SLAB], BF16, tag="srcz")
                        sy.dma_start(srcz_sl[:], srcz_row[lo:lo + SLAB].rearrange("e -> () e"))
                        att_sl = slp.tile([1, SLAB], F32, tag="attsl")

                        rms = []
                        for tt in range(ST):
                            t = s * ST + tt
                            e0, f0 = t * 512, tt * 512
                            prod = tpp.tile([P, 512], BF16, tag="prod")
                            ve.tensor_tensor(prod[:], hsrc_sl[:, f0:f0 + 512],
                                             hr_fm[:, e0:e0 + 512], ALU.mult)
                            mps = lpool.tile([P, 512], F32, tag="msgps", bufs=2)
                            te.matmul(mps[:], msgW[:, k, 0, :], prod[:],
                                      start=True, stop=False)
                            te.matmul(mps[:], msgW[:, k, 1, :], hsrc_sl[:, f0:f0 + 512],
                                      start=False, stop=False)
                            te.matmul(mps[:], msgW[:, k, 3, :], hr_fm[:, e0:e0 + 512],
                                      start=False, stop=False)
                            te.matmul(mps[:], msgW[:, k, 4, :], conf_sl[:, f0:f0 + 512],
                                      start=False, stop=False)
                            te.matmul(mps[:], w3sum[:, k, :], srcz_sl[:, f0:f0 + 512],
                                      start=False, stop=True)
                            rmsg = rmp.tile([P, 512], BF16, tag="rmsg")
                            sc.activation(rmsg[:], mps[:], AF.Relu, bias=msgb[:, k:k + 1])
                            rms.append(rmsg)
                            aps = lpool.tile([1, 512], F32, tag="attps", bufs=2)
                            te.matmul(aps[:], a1[:, k:k + 1], rmsg[:])
                            sc.copy(att_sl[:, f0:f0 + 512], aps[:])

                        # att row -> chunk layout, scalar chain for this slab
                        c0 = s * SC
                        sy.dma_start(att_dr[lo:lo + SLAB].rearrange("e -> () e"),
                                     att_sl[:])
                        sy.dma_start(
                            att_ch[:, c0:c0 + SC].opt(),
                            att_dr[lo:lo + SLAB].rearrange("(c p) -> p c", p=P),
                        )
                        ve.tensor_tensor(att_ch[:, c0:c0 + SC], att_ch[:, c0:c0 + SC],
                                         attrelp[:, k, c0:c0 + SC], ALU.add)
                        lr_t = tpp.tile([P, SC], F32, tag="lrt")
                        ve.tensor_scalar(lr_t[:], att_ch[:, c0:c0 + SC], 0.01, None,
                                         ALU.mult)
                        ve.tensor_tensor(att_ch[:, c0:c0 + SC], att_ch[:, c0:c0 + SC],
                                         lr_t[:], ALU.max)
                        sc.activation(att_ch[:, c0:c0 + SC], att_ch[:, c0:c0 + SC],
                                      AF.Exp)
                        ve.tensor_tensor(att_ch[:, c0:c0 + SC], att_ch[:, c0:c0 + SC],
                                         em_ch[:, c0:c0 + SC], ALU.mult)   # em*exp(att)
                        ve.tensor_copy(exab_ch[:, c0:c0 + SC], att_ch[:, c0:c0 + SC])
                        ve.tensor_tensor(w_ch[:, c0:c0 + SC], att_ch[:, c0:c0 + SC],
                                         s_ch[:, c0:c0 + SC], ALU.mult)

                        # transpose + scale + scatter
                        for tt in range(ST):
                            t = s * ST + tt
                            trp = lpool.tile([P, 4, P], BF16, tag="trps", bufs=1)
                            for j in range(4):
                                te.transpose(trp[:, j, :],
                                             rms[tt][:, 128 * j:128 * j + 128],
                                             ident_b[:])
                            wm = tpp.tile([P, 4, P], BF16, tag="wm")
                            ve.tensor_tensor(
                                wm[:], trp[:],
                                w_ch[:, 4 * t:4 * t + 4].broadcast_to([P, 4, P]),
                                ALU.mult,
                            )
                            for j in range(4):
                                c = 4 * t + j
                                blk = c // CPB
                                ust = c == 0 or c == 4 * CPB
                                usp = c == 4 * CPB - 1 or c == 8 * CPB - 1
                                cl = 128 * (c % SC)
                                te.matmul(usum_ps[:, blk, :],
                                          oh_sl[:, cl:cl + 128],
                                          wm[:, j, :],
                                          start=ust, stop=usp)
                                te.matmul(sm_ps[:, blk:blk + 1],
                                          oh_sl[:, cl:cl + 128],
                                          exab_ch[:, c:c + 1],
                                          start=(c == 0), stop=(c == 8 * CPB - 1))

                  # ---------------- layer tail ----------------
                  if True:
                    with (
                        tc.tile_pool(name=f"tl{k}", bufs=1) as tlp,
                        tc.tile_pool(name=f"tlps{k}", bufs=1, space="PSUM") as tlps,
                    ):
                        sm_s = tlp.tile([P, NBLK], F32)
                        ve.tensor_scalar(sm_s[:], sm_ps[:], 1e-8, None, ALU.add)
                        rsm = tlp.tile([P, NBLK], F32)
                        ve.reciprocal(rsm[:], sm_s[:])
                        aggr = tlp.tile([P, NBLK, P], BF16)
                        ve.tensor_tensor(aggr[:], usum_ps[:],
                                         rsm[:].broadcast_to([P, NBLK, P]), ALU.mult)
                        aggrT = tlp.tile([P, NBLK, P], BF16)
                        trp2 = tlps.tile([P, NBLK, P], BF16, bufs=1, tag="tr2")
                        for bb in range(NBLK):
                            te.transpose(trp2[:, bb, :], aggr[:, bb, :], ident_b[:])
                        ve.tensor_copy(aggrT[:], trp2[:])

                        hb_ps = tlps.tile([P, NBLK, P], F32, tag="hb")
                        for bb in range(NBLK):
                            te.matmul(hb_ps[:, bb, :], aggrT[:, bb, :], updW[:, k, :],
                                      start=True, stop=False)
                            te.matmul(hb_ps[:, bb, :], ones_r1b[:], updb_row[:, k, :],
                                      start=False, stop=True)

                        hs = tlp.tile([P, NBLK, P], F32)
                        ve.tensor_tensor(hs[:], hb_ps[:], h_tiles[k][:], ALU.add)
                        mu = tlp.tile([P, NBLK], F32)
                        ve.tensor_reduce(mu[:], hs[:], mybir.AxisListType.X, ALU.add)
                        ve.tensor_scalar(mu[:], mu[:], 1.0 / P, None, ALU.mult)
                        xc = tlp.tile([P, NBLK, P], F32)
                        ve.tensor_tensor(xc[:], hs[:], mu[:].broadcast_to([P, NBLK, P]),
                                         ALU.subtract)
                        sq = tlp.tile([P, NBLK, P], F32)
                        sc.activation(sq[:], xc[:], AF.Square)
                        var = tlp.tile([P, NBLK], F32)
                        ve.tensor_reduce(var[:], sq[:], mybir.AxisListType.X, ALU.add)
                        ve.tensor_scalar(var[:], var[:], 1.0 / P, None, ALU.mult)
                        sd = tlp.tile([P, NBLK], F32)
                        sc.activation(sd[:], var[:], AF.Sqrt, bias=eps_col[:])
                        rsd = tlp.tile([P, NBLK], F32)
                        ve.reciprocal(rsd[:], sd[:])
                        hn = h_tiles[k + 1]
                        ve.tensor_tensor(hn[:], xc[:], rsd[:].broadcast_to([P, NBLK, P]),
                                         ALU.mult)
                        ve.tensor_tensor(hn[:], hn[:],
                                         g_rep[:].rearrange("p d -> p () d").broadcast_to([P, NBLK, P]),
                                         ALU.mult)
                        ve.tensor_tensor(hn[:], hn[:],
                                         b_rep[:].rearrange("p d -> p () d").broadcast_to([P, NBLK, P]),
                                         ALU.add)

                        sy.dma_start(out_d[k:k + 1, :], hn[0:1, 0, :])

                        if k < L - 1:
                            hstage = tlp.tile([P, NBLK, P], BF16)
                            ve.tensor_copy(hstage[:], hn[:])
                            sy.dma_start(
                                hhalf[:].rearrange("(b p) d -> p b d", p=P),
                                hstage[:],
                            )
                            gp.collective_compute(
                                "AllGather",
                                ALU.bypass,
                                replica_groups=[[0, 1], [2, 3], [4, 5], [6, 7]],
                                ins=[hhalf[:].opt()],
                                outs=[hfull[k % 2][:].opt()],
                            )

    nc.compile()
    return nc


_PROGRAM_CACHE = {}


def _get_program(ecap):
    if ecap not in _PROGRAM_CACHE:
        _PROGRAM_CACHE[ecap] = build_program(ecap)
    return _PROGRAM_CACHE[ecap]


def kernel(**inputs):
    in_maps, ecap = prepare_core_inputs(inputs)
    nc = _get_program(ecap)
    res = run_bass_kernel_spmd(nc, in_maps, list(range(8)))
    outs = np.stack([np.asarray(res.results[2 * b]["out"]) for b in range(B)], axis=0)
    return outs.astype(np.float32)
